# revision 8
# baseline (speedup 1.0000x reference)
"""Trainium2 Bass kernel for decay-masked attention (nn_ACPClassifier).

Reference computation (per batch b):
    x  = Q K^T / sqrt(D)                      [S, S]
    Dm = tril(gamma^(i-j));  Dm /= sqrt(rowsum(Dm))[j]   (column scale)
    x  = x * Dm                               (now lower-triangular)
    v  = max(|rowsum(x)|, 1)                  [S]
    x  = x / v[j]                             (column scale)
    out = x @ V                               [S, D]

Sharding: data-parallel over batch, 1 batch per NeuronCore (B=8, 8 cores).

Per-core kernel strategy:
  * Q, K cast to bf16 via SWDGE DMA (HBM->HBM), then xbar DMA-transposed
    into [d-chunk(128), seq] layout for the TensorEngine.
  * Scores are computed TRANSPOSED: ST[j, i] = sum_d K[j,d] Q[i,d] so the
    PV matmul (contract over j) needs no on-chip transposes at all.
  * The decay mask factorizes: gamma^(i-j)/(32 sqrt(rs[j]))
        = [gamma^(i0-j)/(32 sqrt(rs[j]))]  (per-partition scale, fused into
                                            the PSUM->SBUF score copy)
        * [gamma^(i-i0)]                   (per-row scale, fused into the
                                            output PSUM->SBUF copy and a
                                            [1,512] fixup on the v row)
    where i0 is the query-stripe base. No full [S,S] decay multiply.
  * Lower-triangular sparsity at [128j x 512i] block granularity: 40 of 64
    blocks computed; diagonal blocks get a fused triangular mask.
  * v = rowsum via ones-vector matmuls accumulated into a PSUM row [1,512];
    1/v folded into V rows (V' = V * vinv) -- scales 2M elements instead of
    the 8.9M score elements.
"""

import numpy as np

GAMMA = 0.96875
S, D, P, W = 2048, 1024, 128, 512
NM = S // W          # 4 query stripes of 512 rows
DK = D // P          # 8 contraction chunks of 128
NCORES = 8

_cache: dict = {}


def _pair_index(m: int, c: int) -> int:
    # column index into the colscale table for (stripe m, j-block c)
    return [0, 4, 12, 24][m] + c


def _host_constants():
    import ml_dtypes

    jg = np.arange(S, dtype=np.float64)
    rs = (1.0 - GAMMA ** (jg + 1.0)) / (1.0 - GAMMA)          # rowsum of tril decay
    kfac = 1.0 / (np.sqrt(float(D)) * np.sqrt(rs))            # [S]

    npairs = sum(4 * (m + 1) for m in range(NM))              # 40
    cs = np.zeros((P, npairs), np.float64)
    for m in range(NM):
        for c in range(4 * (m + 1)):
            jl = np.arange(P)
            j = 128 * c + jl
            cs[:, _pair_index(m, c)] = GAMMA ** (512.0 * m - j) * kfac[j]
    colscale = cs.astype(np.float32)

    growrow = (GAMMA ** np.arange(W, dtype=np.float64)).astype(np.float32)[None, :]
    growcol = (
        GAMMA ** (np.arange(P, dtype=np.float64)[:, None] + 128.0 * np.arange(4)[None, :])
    ).astype(np.float32)
    mask = (np.arange(W)[None, :] >= np.arange(P)[:, None]).astype(np.float32)
    ones = np.ones((P, 1), ml_dtypes.bfloat16)
    return colscale, growrow, growcol, mask, ones


def _build(loop_n: int = 1):
    import contextlib

    import concourse.bass as bass
    import concourse.mybir as mybir
    import concourse.tile as tile
    from concourse import bacc

    f32 = mybir.dt.float32
    bf16 = mybir.dt.bfloat16
    mult = mybir.AluOpType.mult

    nc = bacc.Bacc("TRN2", target_bir_lowering=False, debug=False, num_devices=NCORES)

    q_ext = nc.dram_tensor("q", [S, D], f32, kind="ExternalInput")
    k_ext = nc.dram_tensor("k", [S, D], f32, kind="ExternalInput")
    v_ext = nc.dram_tensor("v", [S, D], f32, kind="ExternalInput")
    out_ext = nc.dram_tensor("out", [S, D], f32, kind="ExternalOutput")

    qbf = [nc.dram_tensor(f"qbf{m}", [W, D], bf16) for m in range(NM)]
    kbf = [nc.dram_tensor(f"kbf{m}", [W, D], bf16) for m in range(NM)]

    colscale_np, growrow_np, growcol_np, mask_np, ones_np = _host_constants()
    colscale_d = nc.inline_tensor(colscale_np, "colscale_d")
    growrow_d = nc.inline_tensor(growrow_np, "growrow_d")
    growcol_d = nc.inline_tensor(growcol_np, "growcol_d")
    mask_d = nc.inline_tensor(mask_np, "mask_d")
    ones_d = nc.inline_tensor(ones_np, "ones_d")

    with tile.TileContext(nc) as tc:
        loop_cm = tc.For_i(0, loop_n, 1) if loop_n > 1 else contextlib.nullcontext()
        with (
            loop_cm,
            tc.tile_pool(name="const", bufs=1) as constp,
            tc.tile_pool(name="big", bufs=1) as big,
            tc.tile_pool(name="stall", bufs=2) as stallp,
            tc.tile_pool(name="vstage", bufs=2) as vstagep,
            tc.tile_pool(name="outsb", bufs=3) as outsbp,
            tc.tile_pool(name="vtiny", bufs=2) as vtinyp,
            tc.tile_pool(name="stps", bufs=3, space="PSUM") as stpsp,
            tc.tile_pool(name="outps", bufs=2, space="PSUM") as outpsp,
            tc.tile_pool(name="vrowps", bufs=1, space="PSUM") as vrowpsp,
        ):
            # ---- constants to SBUF ----
            colscale = constp.tile([P, colscale_np.shape[1]], f32, name="colscale")
            growrow = constp.tile([1, W], f32, name="growrow")
            growcol = constp.tile([P, 4], f32, name="growcol")
            mask = constp.tile([P, W], f32, name="mask")
            ones = constp.tile([P, 1], bf16, name="ones")
            nc.sync.dma_start(colscale[:], colscale_d.ap())
            nc.sync.dma_start(growrow[:], growrow_d.ap())
            nc.sync.dma_start(growcol[:], growcol_d.ap())
            nc.sync.dma_start(mask[:], mask_d.ap())
            nc.sync.dma_start(ones[:], ones_d.ap())

            # ---- persistent big tiles ----
            qt = big.tile([P, DK, S], bf16, name="qt")      # qt[p, dk, i] = Q[i, 128*dk+p]
            kt = big.tile([P, DK, S], bf16, name="kt")      # kt[p, dk, j] = K[j, 128*dk+p]
            vp = big.tile([P, S // P, D], bf16, name="vp")  # vp[p, b, d] = V'[128b+p, d]
            vinv_all = big.tile([P, S // P], f32, name="vinv_all")

            # ---- input prep: f32->bf16 cast in DRAM, then xbar transpose ----
            for m in range(NM):
                nc.gpsimd.dma_start(qbf[m][:], q_ext[m * W : (m + 1) * W, :])
                nc.gpsimd.dma_start(kbf[m][:], k_ext[m * W : (m + 1) * W, :])
            for m in range(NM):
                for dk in range(DK):
                    nc.sync.dma_start_transpose(
                        kt[:, dk, m * W : (m + 1) * W],
                        kbf[m][:, dk * P : (dk + 1) * P],
                    )
                for dk in range(DK):
                    nc.sync.dma_start_transpose(
                        qt[:, dk, m * W : (m + 1) * W],
                        qbf[m][:, dk * P : (dk + 1) * P],
                    )

            # ---- main stripe loop ----
            for m in range(NM):
                nblk = 4 * (m + 1)

                # V rows for this stripe
                vstage = vstagep.tile([P, 4, D], f32, name="vstage")
                nc.sync.dma_start(
                    vstage[:],
                    v_ext[m * W : (m + 1) * W, :].rearrange("(b p) d -> p b d", p=P),
                )

                # scores (transposed, scaled, masked) for this stripe
                st_all = stallp.tile([P, 16, W], bf16, name="st_all")

                # QK^T: ST[j, i] blocks
                for c in range(nblk):
                    q_diag = c - 4 * m          # >= 0 inside the diagonal region
                    lo = 128 * q_diag if q_diag > 0 else 0
                    ncols = W - lo
                    i0 = m * W + lo
                    st_ps = stpsp.tile([P, W], f32, name="st_ps")
                    for dk in range(DK):
                        nc.tensor.matmul(
                            st_ps[:, :ncols],
                            lhsT=kt[:, dk, 128 * c : 128 * (c + 1)],
                            rhs=qt[:, dk, i0 : i0 + ncols],
                            start=(dk == 0),
                            stop=(dk == DK - 1),
                        )
                    t = _pair_index(m, c)
                    if q_diag >= 0:
                        # fused triangular mask then column scale
                        nc.vector.tensor_tensor(
                            st_all[:, c, lo:W],
                            st_ps[:, :ncols],
                            mask[:, :ncols],
                            mult,
                        )
                        nc.vector.tensor_scalar_mul(
                            st_all[:, c, lo:W],
                            st_all[:, c, lo:W],
                            scalar1=colscale[:, t : t + 1],
                        )
                    else:
                        nc.vector.tensor_scalar_mul(
                            st_all[:, c, :],
                            st_ps[:],
                            scalar1=colscale[:, t : t + 1],
                        )

                # v row: ones^T @ ST accumulated over blocks -> [1, W]
                vrow_ps = vrowpsp.tile([1, W], f32, name="vrow_ps")
                for c in range(nblk):
                    q_diag = c - 4 * m
                    lo = 128 * q_diag if q_diag > 0 else 0
                    nc.tensor.matmul(
                        vrow_ps[:, lo:W],
                        lhsT=ones[:],
                        rhs=st_all[:, c, lo:W],
                        start=(c == 0),
                        stop=(c == nblk - 1),
                    )

                # v fixups: *gamma^(i-i0), max(|.|,1), reciprocal
                vrow_sb = vtinyp.tile([1, W], f32, name="vrow_sb")
                nc.vector.tensor_tensor(vrow_sb[:], vrow_ps[:], growrow[:], mult)
                nc.scalar.activation(
                    out=vrow_sb[:],
                    in_=vrow_sb[:],
                    func=mybir.ActivationFunctionType.Abs,
                )
                nc.vector.tensor_scalar_max(vrow_sb[:], vrow_sb[:], 1.0)
                vinv_row = vtinyp.tile([1, W], f32, name="vinv_row")
                nc.vector.reciprocal(vinv_row[:], vrow_sb[:])

                # scatter [1, 512] row -> [128, 1] columns of vinv_all
                with nc.allow_non_contiguous_dma(reason="128-elem vinv scatter"):
                    for s in range(4):
                        nc.sync.dma_start(
                            vinv_all[:, 4 * m + s : 4 * m + s + 1],
                            vinv_row[:, 128 * s : 128 * (s + 1)],
                        )

                # V' = V * vinv (per-partition scale), cast to bf16
                for b in range(4):
                    nc.vector.tensor_scalar_mul(
                        vp[:, 4 * m + b, :],
                        vstage[:, b, :],
                        scalar1=vinv_all[:, 4 * m + b : 4 * m + b + 1],
                    )

                # PV: out[i, :] = sum_j ST[j, i] * V'[j, :]
                for s in range(4):
                    ngrp = 4 * m + s + 1
                    op0 = outpsp.tile([P, W], f32, name="op0")
                    op1 = outpsp.tile([P, W], f32, name="op1")
                    for c in range(ngrp):
                        nc.tensor.matmul(
                            op0[:],
                            lhsT=st_all[:, c, 128 * s : 128 * (s + 1)],
                            rhs=vp[:, c, 0:W],
                            start=(c == 0),
                            stop=(c == ngrp - 1),
                            skip_group_check=True,
                        )
                        nc.tensor.matmul(
                            op1[:],
                            lhsT=st_all[:, c, 128 * s : 128 * (s + 1)],
                            rhs=vp[:, c, W:D],
                            start=(c == 0),
                            stop=(c == ngrp - 1),
                            skip_group_check=True,
                        )
                    outsb = outsbp.tile([P, D], f32, name="outsb")
                    nc.scalar.activation(
                        out=outsb[:, 0:W],
                        in_=op0[:],
                        func=mybir.ActivationFunctionType.Copy,
                        scale=growcol[:, s : s + 1],
                    )
                    nc.scalar.activation(
                        out=outsb[:, W:D],
                        in_=op1[:],
                        func=mybir.ActivationFunctionType.Copy,
                        scale=growcol[:, s : s + 1],
                    )
                    r0 = m * W + 128 * s
                    nc.sync.dma_start(out_ext[r0 : r0 + 128, :], outsb[:])

    nc.finalize()
    return nc


def _get_nc():
    if "nc" not in _cache:
        _cache["nc"] = _build()
    return _cache["nc"]


def _session(nc):
    """Compile nc once via bass2jax/PJRT shard_map; return (fn, meta)."""
    import jax
    from jax.experimental.shard_map import shard_map
    from jax.sharding import Mesh, PartitionSpec

    import concourse.mybir as mybir
    from concourse import bass2jax

    bass2jax.install_neuronx_cc_hook()

    partition_name = nc.partition_id_tensor.name if nc.partition_id_tensor else None
    in_names: list = []
    out_names: list = []
    out_avals: list = []
    for alloc in nc.m.functions[0].allocations:
        if not isinstance(alloc, mybir.MemoryLocationSet):
            continue
        name = alloc.memorylocations[0].name
        if alloc.kind == "ExternalInput":
            if name != partition_name:
                in_names.append(name)
        elif alloc.kind == "ExternalOutput":
            out_names.append(name)
            out_avals.append(
                jax.core.ShapedArray(tuple(alloc.tensor_shape), mybir.dt.np(alloc.dtype))
            )
    n_params = len(in_names)
    all_in = list(in_names + out_names)
    if partition_name is not None:
        all_in.append(partition_name)
    all_in = tuple(all_in)
    donate = tuple(range(n_params, n_params + len(out_names)))

    def _body(*args):
        operands = list(args)
        if partition_name is not None:
            operands.append(bass2jax.partition_id_tensor())
        outs = bass2jax._bass_exec_p.bind(
            *operands,
            out_avals=tuple(out_avals),
            in_names=all_in,
            out_names=tuple(out_names),
            lowering_input_output_aliases=(),
            sim_require_finite=True,
            sim_require_nnan=True,
            nc=nc,
        )
        return tuple(outs)

    devices = jax.devices()[:NCORES]
    mesh = Mesh(np.asarray(devices), ("core",))
    nio = n_params + len(out_names)
    fn = jax.jit(
        shard_map(
            _body,
            mesh=mesh,
            in_specs=(PartitionSpec("core"),) * nio,
            out_specs=(PartitionSpec("core"),) * len(out_names),
            check_rep=False,
        ),
        donate_argnums=donate,
        keep_unused=True,
    )
    return fn, in_names, out_avals, mesh


def bench(inputs, n_lo=1, n_hi=33, reps=12, verbose=True):
    """HW time per kernel iteration (ns), via slope between two on-device
    loop counts inside one NEFF (amortizes dispatch + transfer overhead)."""
    import time

    import jax
    import jax.numpy as jnp
    from jax.sharding import NamedSharding, PartitionSpec

    Q = np.ascontiguousarray(np.asarray(inputs["Q"]), dtype=np.float32)
    K = np.ascontiguousarray(np.asarray(inputs["K"]), dtype=np.float32)
    V = np.ascontiguousarray(np.asarray(inputs["V"]), dtype=np.float32)
    concat = {
        "q": Q.reshape(NCORES * S, D),
        "k": K.reshape(NCORES * S, D),
        "v": V.reshape(NCORES * S, D),
    }

    walls = {}
    for n in (n_lo, n_hi):
        key = f"nc_loop{n}"
        if key not in _cache:
            _cache[key] = _build(loop_n=n)
        nc = _cache[key]
        fn, in_names, out_avals, mesh = _session(nc)
        sh = NamedSharding(mesh, PartitionSpec("core"))
        dev_in = [jax.device_put(concat[name], sh) for name in in_names]
        zshape = (NCORES * S, D)
        zfn = jax.jit(lambda: jnp.zeros(zshape, jnp.float32), out_shardings=sh)
        ws = []
        for r in range(reps):
            z = zfn()
            jax.block_until_ready(z)
            t0 = time.perf_counter()
            outs = fn(*dev_in, z)
            jax.block_until_ready(outs)
            ws.append(time.perf_counter() - t0)
        walls[n] = ws
        if verbose:
            print(
                f"loop_n={n}: min={min(ws)*1e3:.3f}ms "
                f"med={sorted(ws)[len(ws)//2]*1e3:.3f}ms all={[f'{w*1e3:.2f}' for w in ws]}"
            )
    slope_min = (min(walls[n_hi]) - min(walls[n_lo])) / (n_hi - n_lo)
    med = lambda xs: sorted(xs)[len(xs) // 2]  # noqa: E731
    slope_med = (med(walls[n_hi]) - med(walls[n_lo])) / (n_hi - n_lo)
    if verbose:
        print(f"slope(min)={slope_min*1e9:.0f}ns slope(med)={slope_med*1e9:.0f}ns")
    return slope_min * 1e9


def run(Q, K, V, trace=False, **kwargs):
    """Run on 8 NeuronCores; returns (out [B,S,D] f32, BassKernelResults)."""
    from concourse.bass_utils import run_bass_kernel_spmd

    Q = np.ascontiguousarray(np.asarray(Q), dtype=np.float32)
    K = np.ascontiguousarray(np.asarray(K), dtype=np.float32)
    V = np.ascontiguousarray(np.asarray(V), dtype=np.float32)
    assert Q.shape == (NCORES, S, D), Q.shape

    nc = _get_nc()
    in_maps = [{"q": Q[i], "k": K[i], "v": V[i]} for i in range(NCORES)]
    res = run_bass_kernel_spmd(
        nc, in_maps, core_ids=list(range(NCORES)), trace=trace, **kwargs
    )
    out = np.stack([res.results[i]["out"] for i in range(NCORES)], axis=0)
    return out.astype(np.float32), res


def kernel(Q, K, V):
    out, _ = run(Q, K, V, trace=False)
    return out
